# revision 37
# baseline (speedup 1.0000x reference)
"""InterferenceAttention Trainium2 kernel (v2).

Full-input contract: kernel(**inputs) takes the unsharded numpy inputs and
returns the full [B, L, D] output. Shards the H=16 heads across 8
NeuronCores (2 heads per core), runs a Bass/Tile kernel SPMD, and reduces
the per-core partial output projections on the host.

v2 structure (per core, h=2 local heads, L=2048, D=1024, hd=64):
  - all inputs host-cast to bf16, weights host-transposed; 1/sqrt(hd)
    folded into Wq/bq; head gate g = sigmoid(gamma)*beta computed on host.
  - x streamed as 8 bf16 chunks; during the stream the PE accumulates
    k (both halves), q (half 0) and the phase projection pT (half 0).
  - post-stream: pT half 1, phase transposes to [l, 4] layout, normalize
    (reciprocal_approx_fast + sqrt) -> aug rows; vT half 0 + PE transposes
    to v tiles [l, hd] layout.
  - attention with q-halves outer, heads inner, scores ST[lk,lq] via the
    66-row augmented contraction (rank-2 interference bias rides in the
    matmul); exp on ACT; A@V with ones-columns for denominators.
  - matmuls sharing a stationary operand skip redundant LDWEIGHTS via
    InstMatmult.ldweights=False (validated bit-exact on HW).
  - vT half 1 / q half 1 / out-proj of half 0 are injected into the
    attention loops to keep the PE queue busy; out-proj half 1 is the tail.
  - per-core partial output written bf16; host sums partials + bo.
"""

import numpy as np

import concourse.bass as bass
import concourse.mybir as mybir
import concourse.tile as tile
from concourse import bacc
from concourse.bass_utils import run_bass_kernel_spmd
from concourse.masks import make_identity

B = 1
L = 2048
D = 1024
H = 16
HD = D // H  # 64
BETA = 0.08

N_CORES = 8
NH = H // N_CORES          # 2 local heads per core
HW = NH * HD               # 128 local head dims per core
LT = L // 128              # 16 l tiles
DT = D // 128              # 8 d chunks

FP32 = mybir.dt.float32
BF16 = mybir.dt.bfloat16
AF = mybir.ActivationFunctionType
ALU = mybir.AluOpType

DEDUPE = True   # skip LDWEIGHTS on matmuls reusing the stationary operand
DEBUG = False   # add intermediate-dump outputs

_NC = None
TRACE = False
LAST_EXEC_NS = None
LAST_RESULTS = None


def _mmg(nc, lhsT, pairs, start, stop):
    """Emit a group of matmuls sharing one stationary lhsT."""
    for out, rhs in pairs:
        nc.tensor.matmul(out, lhsT=lhsT, rhs=rhs, start=start, stop=stop)


def _dedupe_ldweights(nc):
    """Drop InstLdweights whose weights AP matches the previous PE weight
    load (weights stay resident in the PE array across matmuls). Waits on a
    dropped load are merged into the next retained instruction. Runs after
    TileContext legalization (which inserts one load per matmul) and before
    nc.compile()."""
    removed = 0
    for f in nc.m.functions:
        for bb in f.blocks:
            insts = list(bb.instructions)
            last_key = None
            drop = []
            carry = []
            for idx, ins in enumerate(insts):
                if isinstance(ins, mybir.InstLdweights):
                    key = (str(ins.perf_mode), str(ins.is_transpose),
                           str(ins.ins[0]))
                    si = ins.sync_info
                    if key == last_key and not (si is not None and
                                                len(si.on_update) > 0):
                        if si is not None and len(si.on_wait) > 0:
                            carry.append((idx, list(si.on_wait)))
                        drop.append(idx)
                    else:
                        last_key = key
            if not drop:
                continue
            drop_set = set(drop)
            # merge carried waits into the next retained instruction
            for idx, waits in carry:
                j = idx + 1
                while j in drop_set:
                    j += 1
                tgt = insts[j]
                si = tgt.sync_info
                if si is None:
                    tgt.sync_info = mybir.SyncInfo(on_wait=waits, on_update=[])
                else:
                    si.on_wait = list(si.on_wait) + waits
            for idx in sorted(drop_set, reverse=True):
                del bb.instructions[idx]
            removed += len(drop_set)
    return removed


def _build():
    nc = bacc.Bacc("TRN2", target_bir_lowering=False, debug=False)

    xt_d = nc.dram_tensor("xt", [D, L], BF16, kind="ExternalInput").ap()
    # weights host-prechunked: [128, D] with chunk dc at cols dc*128..
    wq_d = nc.dram_tensor("wq", [128, D], BF16, kind="ExternalInput").ap()
    wk_d = nc.dram_tensor("wk", [128, D], BF16, kind="ExternalInput").ap()
    wv_d = nc.dram_tensor("wv", [128, D], BF16, kind="ExternalInput").ap()
    wp_d = nc.dram_tensor("wp", [128, 4 * DT], BF16, kind="ExternalInput").ap()
    wo_d = nc.dram_tensor("wo", [HW, D], BF16, kind="ExternalInput").ap()
    b4_d = nc.dram_tensor("bias4", [128, 4], FP32, kind="ExternalInput").ap()
    g_d = nc.dram_tensor("garr", [128, NH], FP32, kind="ExternalInput").ap()
    out_d = nc.dram_tensor("partial", [L, D], BF16, kind="ExternalOutput").ap()
    dbg = {}
    if DEBUG:
        for nm, shp, dt_ in (("d_ka0", [66, L], BF16), ("d_ka1", [66, L], BF16),
                             ("d_qa0", [66, L], BF16), ("d_qa1", [66, L], BF16),
                             ("d_vt", [128, 192 * LT], BF16),
                             ("d_oT", [128, L], BF16), ("d_pT", [4, L], FP32),
                             ("d_asrc", [128, 128], BF16),
                             ("d_astg", [128, 128], BF16)):
            dbg[nm] = nc.dram_tensor(nm, shp, dt_, kind="ExternalOutput").ap()

    with tile.TileContext(nc) as tc:
        _emit(nc, tc, xt_d, wq_d, wk_d, wv_d, wp_d, wo_d, b4_d, g_d, out_d,
              dbg)
    if DEDUPE:
        _dedupe_ldweights(nc)
    nc.compile()
    return nc


def _emit(nc, tc, xt_d, wq_d, wk_d, wv_d, wp_d, wo_d, b4_d, g_d, out_d,
          dbg=None):
    from contextlib import ExitStack
    ctx = ExitStack()
    const = ctx.enter_context(tc.tile_pool(name="const", bufs=1))
    wpool = ctx.enter_context(tc.tile_pool(name="wpool", bufs=1))
    xp = ctx.enter_context(tc.tile_pool(name="xp", bufs=1))
    qk = ctx.enter_context(tc.tile_pool(name="qk", bufs=1))
    vtp = ctx.enter_context(tc.tile_pool(name="vtp", bufs=1))
    php = ctx.enter_context(tc.tile_pool(name="php", bufs=1))
    expp = ctx.enter_context(tc.tile_pool(name="expp", bufs=1))
    otp = ctx.enter_context(tc.tile_pool(name="otp", bufs=1))
    osb = ctx.enter_context(tc.tile_pool(name="osb", bufs=1))
    ps = ctx.enter_context(tc.tile_pool(name="ps", bufs=1, space="PSUM"))

    # ---- wk (gates the first stream matmul) then x chunks, sync queue ----
    wk_sb = wpool.tile([128, D], BF16)
    nc.sync.dma_start(out=wk_sb, in_=wk_d)
    xT = []
    for dc in range(DT):
        t = xp.tile([128, L], BF16, name=f"xT{dc}")
        nc.sync.dma_start(out=t, in_=xt_d[dc * 128:(dc + 1) * 128, :])
        xT.append(t)

    # ---- remaining weights (gpsimd DMA queue), bf16 + prechunked ----
    wq_sb = wpool.tile([128, D], BF16)
    nc.gpsimd.dma_start(out=wq_sb, in_=wq_d)
    wp_sb = wpool.tile([128, 4 * DT], BF16)
    nc.gpsimd.dma_start(out=wp_sb, in_=wp_d)
    wv_sb = wpool.tile([128, D], BF16)
    nc.gpsimd.dma_start(out=wv_sb, in_=wv_d)

    # ---- constants / small inputs (after the stream-critical DMAs) ----
    b4 = const.tile([128, 4], FP32)
    nc.gpsimd.dma_start(out=b4, in_=b4_d)
    garr = const.tile([128, NH], FP32)
    nc.gpsimd.dma_start(out=garr, in_=g_d)
    wo_sb = wpool.tile([128, D], BF16)
    nc.gpsimd.dma_start(out=wo_sb, in_=wo_d)
    ident_bf = const.tile([128, 128], BF16)
    make_identity(nc, ident_bf)
    ident_f32 = const.tile([128, 128], FP32)
    make_identity(nc, ident_f32)

    # ---- persistent SBUF ----
    qa = [qk.tile([66, L], BF16, name=f"qa{h}") for h in range(NH)]
    ka = [qk.tile([66, L], BF16, name=f"ka{h}") for h in range(NH)]
    # v tiles per lt: [ones | v_h0 | ones | v_h1] so each head's A@V block
    # ([ones | v_h]) puts softmax denominators at partition base 0.
    vt = vtp.tile([128, 256 * LT], BF16, name="vt")
    vt_view = vt.rearrange("p (lt seg) -> p lt seg", seg=256)
    nc.vector.memset(
        vt.rearrange("p (a b) -> p a b", b=128)[:, :, 0:64], 1.0)
    oT_sb = otp.tile([128, L], BF16, name="oT_sb")
    pT_sb = php.tile([4, L], FP32, name="pT_sb")

    # ================= stream phase =================
    # PSUM: kps0(st), kps1(st), q0ps(ot), pT0ps(ot) = 8 banks
    kps = [ps.tile([128, 1024], FP32, tag="st", bufs=2, name=f"kps{c}")
           for c in range(2)]
    q0ps = ps.tile([128, 1024], FP32, tag="ot", bufs=2, name="q0ps")
    pT0ps = ps.tile([4, 1024], FP32, tag="ot", bufs=2, name="pT0ps")
    for dc in range(DT):
        st8 = dict(start=(dc == 0), stop=(dc == DT - 1))
        _mmg(nc, wk_sb[:, dc * 128:(dc + 1) * 128],
             [(kps[cc][:, n * 512:(n + 1) * 512],
               xT[dc][:, cc * 1024 + n * 512: cc * 1024 + (n + 1) * 512])
              for cc in range(2) for n in range(2)], **st8)
        _mmg(nc, wq_sb[:, dc * 128:(dc + 1) * 128],
             [(q0ps[:, n * 512:(n + 1) * 512],
               xT[dc][:, n * 512:(n + 1) * 512]) for n in range(2)], **st8)
        _mmg(nc, wp_sb[:, 4 * dc:4 * dc + 4],
             [(pT0ps[:, n * 512:(n + 1) * 512],
               xT[dc][:, n * 512:(n + 1) * 512]) for n in range(2)], **st8)

    # ---- evacuations (ACT, idle pre-attention): add biases, cast bf16 ----
    # Order matters: kps0 evacs free the st slot for pT1; pT0/pT1 evacs gate
    # the phase transposes (the aug critical path); kps1/q0 only gate the
    # first scores, which start later.
    for cc in range(2):
        for h in range(NH):
            nc.scalar.activation(
                out=ka[h][0:HD, cc * 1024:(cc + 1) * 1024],
                in_=kps[cc][h * HD:(h + 1) * HD, :],
                func=AF.Identity, bias=b4[h * HD:(h + 1) * HD, 1:2], scale=1.0)
    nc.scalar.activation(out=pT_sb[0:4, 0:1024], in_=pT0ps,
                         func=AF.Identity, bias=b4[0:4, 3:4], scale=1.0)

    # ---- pT half 1 (PE; st slot after kps0 freed) ----
    pT1ps = ps.tile([4, 1024], FP32, tag="st", bufs=2, name="pT1ps")
    for dc in range(DT):
        _mmg(nc, wp_sb[:, 4 * dc:4 * dc + 4],
             [(pT1ps[:, n * 512:(n + 1) * 512],
               xT[dc][:, 1024 + n * 512:1024 + (n + 1) * 512])
              for n in range(2)],
             start=(dc == 0), stop=(dc == DT - 1))
    nc.scalar.activation(out=pT_sb[0:4, 1024:2048], in_=pT1ps,
                         func=AF.Identity, bias=b4[0:4, 3:4], scale=1.0)
    for h in range(NH):
        nc.scalar.activation(
            out=qa[h][0:HD, 0:1024], in_=q0ps[h * HD:(h + 1) * HD, :],
            func=AF.Identity, bias=b4[h * HD:(h + 1) * HD, 0:1], scale=1.0)

    # ---- vT half 0 (PE) before the phase transposes so the DVE work
    # (normalize chain) overlaps PE instead of serializing before it ----
    vT0ps = ps.tile([128, 1024], FP32, tag="st", bufs=2, name="vT0ps")
    for dc in range(DT):
        _mmg(nc, wv_sb[:, dc * 128:(dc + 1) * 128],
             [(vT0ps[:, n * 512:(n + 1) * 512],
               xT[dc][:, n * 512:(n + 1) * 512]) for n in range(2)],
             start=(dc == 0), stop=(dc == DT - 1))
    vsb0 = vtp.tile([128, 1024], BF16, tag="vsb", bufs=2, name="vsb0")
    nc.vector.tensor_scalar(out=vsb0, in0=vT0ps, scalar1=b4[:, 2:3],
                            scalar2=None, op0=ALU.add)

    # ---- phase transpose: [4, L] -> [128, 4*LT] (col 4*lt+r) ----
    phtr = ps.tile([128, 1024], FP32, tag="st", bufs=2, name="phtr")
    for lt in range(LT):
        nc.tensor.transpose(out=phtr[:, 4 * lt:4 * lt + 4],
                            in_=pT_sb[0:4, lt * 128:(lt + 1) * 128],
                            identity=ident_f32[0:4, 0:4])
    pn = php.tile([128, 4 * LT], FP32, name="pn")
    nc.vector.tensor_copy(out=pn, in_=phtr[:, 0:4 * LT])

    # ---- normalize + gate -> aug_src [128, 128] bf16 ----
    # aug_src col blocks of 16 (one col per lt):
    #   0:c0 1:s0 2:g*c0 3:g*s0 4:c1 5:s1 6:g*c1 7:g*s1
    sq = php.tile([128, 4 * LT], FP32, name="sq")
    nc.vector.tensor_tensor(out=sq, in0=pn, in1=pn, op=ALU.mult)
    aug_src = php.tile([128, 128], BF16, name="aug_src")
    for h in range(NH):
        n2 = php.tile([128, LT], FP32, tag="n2", bufs=2, name=f"n2_{h}")
        nc.vector.tensor_tensor(out=n2, in0=sq[:, 2 * h::4],
                                in1=sq[:, 2 * h + 1::4], op=ALU.add)
        nc.vector.tensor_scalar_max(n2, n2, 1e-12)
        rf = php.tile([128, LT], FP32, tag="rf", bufs=2, name=f"rf_{h}")
        nc.vector.reciprocal_approx_fast(out=rf, in_=n2)
        rs = php.tile([128, LT], FP32, tag="rs", bufs=2, name=f"rs_{h}")
        nc.scalar.activation(out=rs, in_=rf, func=AF.Sqrt)  # 1/||ph||
        cb = aug_src[:, (4 * h + 0) * 16:(4 * h + 1) * 16]
        sb_ = aug_src[:, (4 * h + 1) * 16:(4 * h + 2) * 16]
        nc.vector.tensor_tensor(out=cb, in0=pn[:, 2 * h::4], in1=rs,
                                op=ALU.mult)
        nc.vector.tensor_tensor(out=sb_, in0=pn[:, 2 * h + 1::4], in1=rs,
                                op=ALU.mult)
        nc.vector.tensor_scalar_mul(
            aug_src[:, (4 * h + 2) * 16:(4 * h + 3) * 16], cb, garr[:, h:h + 1])
        nc.vector.tensor_scalar_mul(
            aug_src[:, (4 * h + 3) * 16:(4 * h + 4) * 16], sb_, garr[:, h:h + 1])

    # ---- vT0 transposes then aug transpose (one bf16 psum tile) ----
    trb = ps.tile([128, 2048], BF16, tag="st", bufs=2, name="trb")
    for j in range(8):
        nc.tensor.transpose(out=trb[:, j * 128:(j + 1) * 128],
                            in_=vsb0[:, j * 128:(j + 1) * 128],
                            identity=ident_bf)
    trv = trb.rearrange("p (j k) -> p j k", k=128)
    nc.vector.tensor_copy(out=vt_view[:, 0:8, 64:128], in_=trv[:, 0:8, 0:64])
    nc.vector.tensor_copy(out=vt_view[:, 0:8, 192:256], in_=trv[:, 0:8, 64:128])
    nc.tensor.transpose(out=trb[:, 1024:1152], in_=aug_src, identity=ident_bf)
    aug_stage = php.tile([128, 128], BF16, name="aug_stage")
    nc.vector.tensor_copy(out=aug_stage, in_=trb[:, 1024:1152])
    aug_rows = ((ka[0], 64), (ka[0], 65), (qa[0], 64), (qa[0], 65),
                (ka[1], 64), (ka[1], 65), (qa[1], 64), (qa[1], 65))
    for v, (tgt, row) in enumerate(aug_rows):
        q = nc.sync if v % 2 == 0 else nc.gpsimd
        q.dma_start(
            out=tgt[row:row + 1, :].rearrange("a (b c) -> a b c", b=LT),
            in_=aug_stage[v * 16:(v + 1) * 16, :])

    # ================= attention =================
    # ot-tag rotation: q0ps(1) pT0ps(2) oT00(1) vT1ps(2) oT01(1) q1ps(2)
    #                  oT10(1) opc0(2) oT11(1) opc1(2)
    vT1ps = None
    vsb1 = None
    q1ps = None
    opc0 = None
    trb2 = None

    def outproj_tile(c, lt, opps, tail):
        col = (c * 8 + lt) * 128
        _mmg(nc, oT_sb[:, col:col + 128],
             [(opps[:, n * 512:(n + 1) * 512],
               wo_sb[:, n * 512:(n + 1) * 512]) for n in range(2)],
             start=True, stop=True)
        ob = osb.tile([128, D], BF16, tag="osb", bufs=4, name=f"ob{c}{lt}")
        if tail:
            # post-attention: ACT is free — split the evac across engines
            nc.scalar.activation(out=ob[:, 0:512], in_=opps[:, 0:512],
                                 func=AF.Copy)
            nc.vector.tensor_copy(out=ob[:, 512:1024], in_=opps[:, 512:1024])
        else:
            nc.vector.tensor_copy(out=ob, in_=opps)
        nc.sync.dma_start(out=out_d[col:col + 128, :], in_=ob)

    for c, h in ((0, 0), (0, 1), (1, 0), (1, 1)):
        oTps = ps.tile([128, 1024], FP32, tag="ot", bufs=2, name=f"oT{c}{h}")
        lo = 128 * h   # head block [ones | v_h] within the 256-wide lt group
        deferred_av = []
        for lk in range(LT):
            stps = ps.tile([128, 1024], FP32, tag="st", bufs=2,
                           name=f"st{c}{h}{lk}")
            _mmg(nc, ka[h][0:66, lk * 128:(lk + 1) * 128],
                 [(stps[:, n * 512:(n + 1) * 512],
                   qa[h][0:66, c * 1024 + n * 512: c * 1024 + (n + 1) * 512])
                  for n in range(2)], start=True, stop=True)
            ex = expp.tile([128, 1024], BF16, tag="exp", bufs=4,
                           name=f"ex{c}{h}{lk}")
            nc.scalar.activation(out=ex, in_=stps, func=AF.Exp)

            def emit_av(lk, ex):
                _mmg(nc, vt[:, lk * 256 + lo: lk * 256 + lo + 128],
                     [(oTps[:, n * 512:(n + 1) * 512],
                       ex[:, n * 512:(n + 1) * 512]) for n in range(2)],
                     start=(lk == 0), stop=(lk == LT - 1))

            if (c, h) == (0, 0) and lk < 3:
                # the v tiles arrive slightly after the aug rows: let the
                # first scores/exps run ahead, flush these A@V at lk 3
                deferred_av.append((lk, ex))
            else:
                for a_lk, a_ex in deferred_av:
                    emit_av(a_lk, a_ex)
                deferred_av = []
                emit_av(lk, ex)

            # -------- injected work --------
            if (c, h) == (0, 0):
                if lk < 4:   # vT half 1: two d-chunks per lk
                    if lk == 0:
                        vT1ps = ps.tile([128, 1024], FP32, tag="ot", bufs=2,
                                        name="vT1ps")
                    for dc in (2 * lk, 2 * lk + 1):
                        _mmg(nc, wv_sb[:, dc * 128:(dc + 1) * 128],
                             [(vT1ps[:, n * 512:(n + 1) * 512],
                               xT[dc][:, 1024 + n * 512:1024 + (n + 1) * 512])
                              for n in range(2)],
                             start=(dc == 0), stop=(dc == DT - 1))
                elif lk == 4:
                    vsb1 = vtp.tile([128, 1024], BF16, tag="vsb", bufs=2,
                                    name="vsb1")
                    nc.vector.tensor_scalar(out=vsb1, in0=vT1ps,
                                            scalar1=b4[:, 2:3], scalar2=None,
                                            op0=ALU.add)
                elif lk in (5, 6):
                    if lk == 5:
                        trb2 = ps.tile([128, 2048], BF16, tag="st", bufs=2,
                                       name="trb2")
                    js = range(0, 4) if lk == 5 else range(4, 8)
                    for j in js:
                        nc.tensor.transpose(
                            out=trb2[:, j * 128:(j + 1) * 128],
                            in_=vsb1[:, j * 128:(j + 1) * 128],
                            identity=ident_bf)
                    if lk == 6:
                        trv2 = trb2.rearrange("p (j k) -> p j k", k=128)
                        nc.vector.tensor_copy(out=vt_view[:, 8:16, 64:128],
                                              in_=trv2[:, 0:8, 0:64])
                        nc.vector.tensor_copy(out=vt_view[:, 8:16, 192:256],
                                              in_=trv2[:, 0:8, 64:128])
            elif (c, h) == (0, 1):
                if lk < 8:   # q half 1: one d-chunk per lk
                    if lk == 0:
                        q1ps = ps.tile([128, 1024], FP32, tag="ot", bufs=2,
                                       name="q1ps")
                    _mmg(nc, wq_sb[:, lk * 128:(lk + 1) * 128],
                         [(q1ps[:, n * 512:(n + 1) * 512],
                           xT[lk][:, 1024 + n * 512:1024 + (n + 1) * 512])
                          for n in range(2)],
                         start=(lk == 0), stop=(lk == DT - 1))
                elif lk in (8, 9):
                    hh = lk - 8
                    nc.vector.tensor_scalar(
                        out=qa[hh][0:HD, 1024:2048],
                        in0=q1ps[hh * HD:(hh + 1) * HD, :],
                        scalar1=b4[hh * HD:(hh + 1) * HD, 0:1], scalar2=None,
                        op0=ALU.add)
            elif (c, h) == (1, 0):
                if lk % 2 == 1:   # out-proj of half 0, one l-tile per 2 lk
                    if lk == 1:
                        opc0 = ps.tile([128, 1024], FP32, tag="ot", bufs=2,
                                       name="opc0")
                    outproj_tile(0, (lk - 1) // 2, opc0, tail=False)

        # -------- drain: softmax normalize into oT_sb (bf16) --------
        # reciprocal_approx_fast is a custom DVE op and silently misreads
        # partition-shifted operands: keep it at partition base 0 (the vt
        # layout puts denominator rows at psum base 0 for both heads).
        rv = otp.tile([64, 1024], FP32, tag="rv", bufs=2, name=f"rv{c}{h}")
        nc.vector.reciprocal_approx_fast(out=rv, in_=oTps[0:64, :])
        nc.vector.tensor_tensor(
            out=oT_sb[h * 64:(h + 1) * 64, c * 1024:(c + 1) * 1024],
            in0=oTps[64:128, :], in1=rv, op=ALU.mult)

    # ---- tail: out-proj of half 1 (two tiles, pipelined) ----
    opc1a = ps.tile([128, 1024], FP32, tag="ot", bufs=2, name="opc1a")
    opc1b = ps.tile([128, 1024], FP32, tag="ot", bufs=2, name="opc1b")
    for lt in range(8):
        outproj_tile(1, lt, opc1a if lt % 2 == 0 else opc1b, tail=True)

    if dbg:
        nc.sync.dma_start(out=dbg["d_ka0"], in_=ka[0])
        nc.sync.dma_start(out=dbg["d_ka1"], in_=ka[1])
        nc.sync.dma_start(out=dbg["d_qa0"], in_=qa[0])
        nc.sync.dma_start(out=dbg["d_qa1"], in_=qa[1])
        nc.sync.dma_start(out=dbg["d_vt"], in_=vt)
        nc.sync.dma_start(out=dbg["d_oT"], in_=oT_sb)
        nc.sync.dma_start(out=dbg["d_pT"], in_=pT_sb)
        nc.sync.dma_start(out=dbg["d_asrc"], in_=aug_src)
        nc.sync.dma_start(out=dbg["d_astg"], in_=aug_stage)
    ctx.close()


def _get_nc():
    global _NC
    if _NC is None:
        _NC = _build()
    return _NC


def kernel(x, Wq, bq, Wk, bk, Wv, bv, Wo, bo, Wp, bp, gamma):
    global LAST_EXEC_NS, LAST_RESULTS
    import ml_dtypes
    BF = ml_dtypes.bfloat16
    nc = _get_nc()

    x2 = np.asarray(x, np.float32).reshape(L, D)
    xt = np.ascontiguousarray(x2.T).astype(BF)
    Wq = np.asarray(Wq, np.float32)
    Wk = np.asarray(Wk, np.float32)
    Wv = np.asarray(Wv, np.float32)
    Wo = np.asarray(Wo, np.float32)
    Wp = np.asarray(Wp, np.float32)
    bq_ = np.asarray(bq, np.float32)
    bk_ = np.asarray(bk, np.float32)
    bv_ = np.asarray(bv, np.float32)
    bp_ = np.asarray(bp, np.float32)
    g_all = (1.0 / (1.0 + np.exp(-np.asarray(gamma, np.float32)))) * BETA
    sc = 1.0 / np.sqrt(HD)

    in_maps = []
    for c in range(N_CORES):
        hs = slice(c * HW, (c + 1) * HW)
        p4 = slice(c * 2 * NH, (c + 1) * 2 * NH)
        bias4 = np.zeros((128, 4), np.float32)
        bias4[:, 0] = bq_[hs] * sc
        bias4[:, 1] = bk_[hs]
        bias4[:, 2] = bv_[hs]
        bias4[0:2 * NH, 3] = bp_[p4]
        garr = np.repeat(g_all[c * NH:(c + 1) * NH][None, :], 128, axis=0)

        def chunk(wT):   # [D, E] -> [128, DT*E]: chunk dc at cols dc*E..
            e = wT.shape[1]
            return np.ascontiguousarray(
                wT.reshape(DT, 128, e).transpose(1, 0, 2).reshape(128, DT * e)
            ).astype(BF)

        in_maps.append({
            "xt": xt,
            "wq": chunk((Wq[hs] * sc).T),
            "wk": chunk(Wk[hs].T),
            "wv": chunk(Wv[hs].T),
            "wp": chunk(Wp[p4].T),
            "wo": np.ascontiguousarray(Wo[:, hs].T).astype(BF),
            "bias4": bias4,
            "garr": np.ascontiguousarray(garr),
        })
    res = run_bass_kernel_spmd(nc, in_maps, list(range(N_CORES)), trace=TRACE)
    LAST_EXEC_NS = res.exec_time_ns
    LAST_RESULTS = res
    acc = np.zeros((L, D), np.float32)
    for c in range(N_CORES):
        acc += np.asarray(res.results[c]["partial"], dtype=np.float32)
    acc += np.asarray(bo, np.float32)[None, :]
    return acc.reshape(B, L, D)


# revision 39
# speedup vs baseline: 1.0175x; 1.0175x over previous
"""InterferenceAttention Trainium2 kernel (v2).

Full-input contract: kernel(**inputs) takes the unsharded numpy inputs and
returns the full [B, L, D] output. Shards the H=16 heads across 8
NeuronCores (2 heads per core), runs a Bass/Tile kernel SPMD, and reduces
the per-core partial output projections on the host.

v2 structure (per core, h=2 local heads, L=2048, D=1024, hd=64):
  - all inputs host-cast to bf16, weights host-transposed; 1/sqrt(hd)
    folded into Wq/bq; head gate g = sigmoid(gamma)*beta computed on host.
  - x streamed as 8 bf16 chunks; during the stream the PE accumulates
    k (both halves), q (half 0) and the phase projection pT (half 0).
  - post-stream: pT half 1, phase transposes to [l, 4] layout, normalize
    (reciprocal_approx_fast + sqrt) -> aug rows; vT half 0 + PE transposes
    to v tiles [l, hd] layout.
  - attention with q-halves outer, heads inner, scores ST[lk,lq] via the
    66-row augmented contraction (rank-2 interference bias rides in the
    matmul); exp on ACT; A@V with ones-columns for denominators.
  - matmuls sharing a stationary operand skip redundant LDWEIGHTS via
    InstMatmult.ldweights=False (validated bit-exact on HW).
  - vT half 1 / q half 1 / out-proj of half 0 are injected into the
    attention loops to keep the PE queue busy; out-proj half 1 is the tail.
  - per-core partial output written bf16; host sums partials + bo.
"""

import numpy as np

import concourse.bass as bass
import concourse.mybir as mybir
import concourse.tile as tile
from concourse import bacc
from concourse.bass_utils import run_bass_kernel_spmd
from concourse.masks import make_identity

B = 1
L = 2048
D = 1024
H = 16
HD = D // H  # 64
BETA = 0.08

N_CORES = 8
NH = H // N_CORES          # 2 local heads per core
HW = NH * HD               # 128 local head dims per core
LT = L // 128              # 16 l tiles
DT = D // 128              # 8 d chunks

FP32 = mybir.dt.float32
BF16 = mybir.dt.bfloat16
AF = mybir.ActivationFunctionType
ALU = mybir.AluOpType

DEDUPE = True   # skip LDWEIGHTS on matmuls reusing the stationary operand
DEBUG = False   # add intermediate-dump outputs

_NC = None
TRACE = False
LAST_EXEC_NS = None
LAST_RESULTS = None


def _mmg(nc, lhsT, pairs, start, stop):
    """Emit a group of matmuls sharing one stationary lhsT."""
    for out, rhs in pairs:
        nc.tensor.matmul(out, lhsT=lhsT, rhs=rhs, start=start, stop=stop)


def _dedupe_ldweights(nc):
    """Drop InstLdweights whose weights AP matches the previous PE weight
    load (weights stay resident in the PE array across matmuls). Waits on a
    dropped load are merged into the next retained instruction. Runs after
    TileContext legalization (which inserts one load per matmul) and before
    nc.compile()."""
    removed = 0
    for f in nc.m.functions:
        for bb in f.blocks:
            insts = list(bb.instructions)
            last_key = None
            drop = []
            carry = []
            for idx, ins in enumerate(insts):
                if isinstance(ins, mybir.InstLdweights):
                    key = (str(ins.perf_mode), str(ins.is_transpose),
                           str(ins.ins[0]))
                    si = ins.sync_info
                    if key == last_key and not (si is not None and
                                                len(si.on_update) > 0):
                        if si is not None and len(si.on_wait) > 0:
                            carry.append((idx, list(si.on_wait)))
                        drop.append(idx)
                    else:
                        last_key = key
            if not drop:
                continue
            drop_set = set(drop)
            # merge carried waits into the next retained instruction
            for idx, waits in carry:
                j = idx + 1
                while j in drop_set:
                    j += 1
                tgt = insts[j]
                si = tgt.sync_info
                if si is None:
                    tgt.sync_info = mybir.SyncInfo(on_wait=waits, on_update=[])
                else:
                    si.on_wait = list(si.on_wait) + waits
            for idx in sorted(drop_set, reverse=True):
                del bb.instructions[idx]
            removed += len(drop_set)
    return removed


def _build():
    nc = bacc.Bacc("TRN2", target_bir_lowering=False, debug=False)

    xt_d = nc.dram_tensor("xt", [D, L], BF16, kind="ExternalInput").ap()
    # weights host-prechunked: [128, D] with chunk dc at cols dc*128..
    wq_d = nc.dram_tensor("wq", [128, D], BF16, kind="ExternalInput").ap()
    wk_d = nc.dram_tensor("wk", [128, D], BF16, kind="ExternalInput").ap()
    wv_d = nc.dram_tensor("wv", [128, D], BF16, kind="ExternalInput").ap()
    wp_d = nc.dram_tensor("wp", [128, 4 * DT], BF16, kind="ExternalInput").ap()
    wo_d = nc.dram_tensor("wo", [HW, D], BF16, kind="ExternalInput").ap()
    b4_d = nc.dram_tensor("bias4", [128, 4], FP32, kind="ExternalInput").ap()
    g_d = nc.dram_tensor("garr", [128, NH], FP32, kind="ExternalInput").ap()
    out_d = nc.dram_tensor("partial", [L, D], BF16, kind="ExternalOutput").ap()
    dbg = {}
    if DEBUG:
        for nm, shp, dt_ in (("d_ka0", [66, L], BF16), ("d_ka1", [66, L], BF16),
                             ("d_qa0", [66, L], BF16), ("d_qa1", [66, L], BF16),
                             ("d_vt", [128, 192 * LT], BF16),
                             ("d_oT", [128, L], BF16), ("d_pT", [4, L], FP32),
                             ("d_asrc", [128, 128], BF16),
                             ("d_astg", [128, 128], BF16)):
            dbg[nm] = nc.dram_tensor(nm, shp, dt_, kind="ExternalOutput").ap()

    with tile.TileContext(nc) as tc:
        _emit(nc, tc, xt_d, wq_d, wk_d, wv_d, wp_d, wo_d, b4_d, g_d, out_d,
              dbg)
    if DEDUPE:
        _dedupe_ldweights(nc)
    nc.compile()
    return nc


def _emit(nc, tc, xt_d, wq_d, wk_d, wv_d, wp_d, wo_d, b4_d, g_d, out_d,
          dbg=None):
    from contextlib import ExitStack
    ctx = ExitStack()
    const = ctx.enter_context(tc.tile_pool(name="const", bufs=1))
    wpool = ctx.enter_context(tc.tile_pool(name="wpool", bufs=1))
    xp = ctx.enter_context(tc.tile_pool(name="xp", bufs=1))
    qk = ctx.enter_context(tc.tile_pool(name="qk", bufs=1))
    vtp = ctx.enter_context(tc.tile_pool(name="vtp", bufs=1))
    php = ctx.enter_context(tc.tile_pool(name="php", bufs=1))
    expp = ctx.enter_context(tc.tile_pool(name="expp", bufs=1))
    otp = ctx.enter_context(tc.tile_pool(name="otp", bufs=1))
    osb = ctx.enter_context(tc.tile_pool(name="osb", bufs=1))
    ps = ctx.enter_context(tc.tile_pool(name="ps", bufs=1, space="PSUM"))

    # ---- wk (gates the first stream matmul) then x chunks, sync queue ----
    wk_sb = wpool.tile([128, D], BF16)
    nc.sync.dma_start(out=wk_sb, in_=wk_d)
    xT = []
    for dc in range(DT):
        t = xp.tile([128, L], BF16, name=f"xT{dc}")
        nc.sync.dma_start(out=t, in_=xt_d[dc * 128:(dc + 1) * 128, :])
        xT.append(t)

    # ---- remaining weights (gpsimd DMA queue), bf16 + prechunked ----
    wq_sb = wpool.tile([128, D], BF16)
    nc.gpsimd.dma_start(out=wq_sb, in_=wq_d)
    wp_sb = wpool.tile([128, 4 * DT], BF16)
    nc.gpsimd.dma_start(out=wp_sb, in_=wp_d)
    wv_sb = wpool.tile([128, D], BF16)
    nc.gpsimd.dma_start(out=wv_sb, in_=wv_d)

    # ---- constants / small inputs (after the stream-critical DMAs) ----
    b4 = const.tile([128, 4], FP32)
    nc.gpsimd.dma_start(out=b4, in_=b4_d)
    garr = const.tile([128, NH], FP32)
    nc.gpsimd.dma_start(out=garr, in_=g_d)
    wo_sb = wpool.tile([128, D], BF16)
    nc.gpsimd.dma_start(out=wo_sb, in_=wo_d)
    ident_bf = const.tile([128, 128], BF16)
    make_identity(nc, ident_bf)
    ident_f32 = const.tile([128, 128], FP32)
    make_identity(nc, ident_f32)

    # ---- persistent SBUF ----
    qa = [qk.tile([66, L], BF16, name=f"qa{h}") for h in range(NH)]
    ka = [qk.tile([66, L], BF16, name=f"ka{h}") for h in range(NH)]
    # v tiles per lt: [ones | v_h0 | ones | v_h1] so each head's A@V block
    # ([ones | v_h]) puts softmax denominators at partition base 0.
    vt = vtp.tile([128, 256 * LT], BF16, name="vt")
    vt_view = vt.rearrange("p (lt seg) -> p lt seg", seg=256)
    nc.vector.memset(
        vt.rearrange("p (a b) -> p a b", b=128)[:, :, 0:64], 1.0)
    oT_sb = otp.tile([128, L], BF16, name="oT_sb")
    pT_sb = php.tile([4, L], FP32, name="pT_sb")

    # ================= stream phase =================
    # PSUM: kps0(st), kps1(st), q0ps(ot), pT0ps(ot) = 8 banks
    kps = [ps.tile([128, 1024], FP32, tag="st", bufs=2, name=f"kps{c}")
           for c in range(2)]
    q0ps = ps.tile([128, 1024], FP32, tag="ot", bufs=2, name="q0ps")
    pT0ps = ps.tile([4, 1024], FP32, tag="ot", bufs=2, name="pT0ps")
    for dc in range(DT):
        st8 = dict(start=(dc == 0), stop=(dc == DT - 1))
        _mmg(nc, wk_sb[:, dc * 128:(dc + 1) * 128],
             [(kps[cc][:, n * 512:(n + 1) * 512],
               xT[dc][:, cc * 1024 + n * 512: cc * 1024 + (n + 1) * 512])
              for cc in range(2) for n in range(2)], **st8)
        _mmg(nc, wq_sb[:, dc * 128:(dc + 1) * 128],
             [(q0ps[:, n * 512:(n + 1) * 512],
               xT[dc][:, n * 512:(n + 1) * 512]) for n in range(2)], **st8)
        _mmg(nc, wp_sb[:, 4 * dc:4 * dc + 4],
             [(pT0ps[:, n * 512:(n + 1) * 512],
               xT[dc][:, n * 512:(n + 1) * 512]) for n in range(2)], **st8)

    # ---- evacuations (ACT, idle pre-attention): add biases, cast bf16 ----
    # Order matters: kps0 evacs free the st slot for pT1; pT0/pT1 evacs gate
    # the phase transposes (the aug critical path); kps1/q0 only gate the
    # first scores, which start later.
    for cc in range(2):
        for h in range(NH):
            nc.scalar.activation(
                out=ka[h][0:HD, cc * 1024:(cc + 1) * 1024],
                in_=kps[cc][h * HD:(h + 1) * HD, :],
                func=AF.Identity, bias=b4[h * HD:(h + 1) * HD, 1:2], scale=1.0)
    nc.scalar.activation(out=pT_sb[0:4, 0:1024], in_=pT0ps,
                         func=AF.Identity, bias=b4[0:4, 3:4], scale=1.0)

    # ---- pT half 1 (PE; st slot after kps0 freed) ----
    pT1ps = ps.tile([4, 1024], FP32, tag="st", bufs=2, name="pT1ps")
    for dc in range(DT):
        _mmg(nc, wp_sb[:, 4 * dc:4 * dc + 4],
             [(pT1ps[:, n * 512:(n + 1) * 512],
               xT[dc][:, 1024 + n * 512:1024 + (n + 1) * 512])
              for n in range(2)],
             start=(dc == 0), stop=(dc == DT - 1))
    nc.scalar.activation(out=pT_sb[0:4, 1024:2048], in_=pT1ps,
                         func=AF.Identity, bias=b4[0:4, 3:4], scale=1.0)
    for h in range(NH):
        nc.scalar.activation(
            out=qa[h][0:HD, 0:1024], in_=q0ps[h * HD:(h + 1) * HD, :],
            func=AF.Identity, bias=b4[h * HD:(h + 1) * HD, 0:1], scale=1.0)

    # ---- vT half 0 (PE) before the phase transposes so the DVE work
    # (normalize chain) overlaps PE instead of serializing before it ----
    vT0ps = ps.tile([128, 1024], FP32, tag="st", bufs=2, name="vT0ps")
    for dc in range(DT):
        _mmg(nc, wv_sb[:, dc * 128:(dc + 1) * 128],
             [(vT0ps[:, n * 512:(n + 1) * 512],
               xT[dc][:, n * 512:(n + 1) * 512]) for n in range(2)],
             start=(dc == 0), stop=(dc == DT - 1))
    vsb0 = vtp.tile([128, 1024], BF16, tag="vsb", bufs=2, name="vsb0")
    nc.vector.tensor_scalar(out=vsb0, in0=vT0ps, scalar1=b4[:, 2:3],
                            scalar2=None, op0=ALU.add)

    # ---- phase transpose: [4, L] -> [128, 4*LT] (col 4*lt+r) ----
    phtr = ps.tile([128, 1024], FP32, tag="st", bufs=2, name="phtr")
    for lt in range(LT):
        nc.tensor.transpose(out=phtr[:, 4 * lt:4 * lt + 4],
                            in_=pT_sb[0:4, lt * 128:(lt + 1) * 128],
                            identity=ident_f32[0:4, 0:4])
    pn = php.tile([128, 4 * LT], FP32, name="pn")
    nc.vector.tensor_copy(out=pn, in_=phtr[:, 0:4 * LT])

    # ---- normalize + gate -> aug_src [128, 128] bf16 ----
    # aug_src col blocks of 16 (one col per lt):
    #   0:c0 1:s0 2:g*c0 3:g*s0 4:c1 5:s1 6:g*c1 7:g*s1
    sq = php.tile([128, 4 * LT], FP32, name="sq")
    nc.vector.tensor_tensor(out=sq, in0=pn, in1=pn, op=ALU.mult)
    aug_src = php.tile([128, 128], BF16, name="aug_src")
    for h in range(NH):
        n2 = php.tile([128, LT], FP32, tag="n2", bufs=2, name=f"n2_{h}")
        nc.vector.tensor_tensor(out=n2, in0=sq[:, 2 * h::4],
                                in1=sq[:, 2 * h + 1::4], op=ALU.add)
        nc.vector.tensor_scalar_max(n2, n2, 1e-12)
        rf = php.tile([128, LT], FP32, tag="rf", bufs=2, name=f"rf_{h}")
        nc.vector.reciprocal_approx_fast(out=rf, in_=n2)
        rs = php.tile([128, LT], FP32, tag="rs", bufs=2, name=f"rs_{h}")
        nc.scalar.activation(out=rs, in_=rf, func=AF.Sqrt)  # 1/||ph||
        cb = aug_src[:, (4 * h + 0) * 16:(4 * h + 1) * 16]
        sb_ = aug_src[:, (4 * h + 1) * 16:(4 * h + 2) * 16]
        nc.vector.tensor_tensor(out=cb, in0=pn[:, 2 * h::4], in1=rs,
                                op=ALU.mult)
        nc.vector.tensor_tensor(out=sb_, in0=pn[:, 2 * h + 1::4], in1=rs,
                                op=ALU.mult)
        nc.vector.tensor_scalar_mul(
            aug_src[:, (4 * h + 2) * 16:(4 * h + 3) * 16], cb, garr[:, h:h + 1])
        nc.vector.tensor_scalar_mul(
            aug_src[:, (4 * h + 3) * 16:(4 * h + 4) * 16], sb_, garr[:, h:h + 1])

    # ---- vT0 transposes then aug transpose (one bf16 psum tile) ----
    trb = ps.tile([128, 2048], BF16, tag="st", bufs=2, name="trb")
    for j in range(8):
        nc.tensor.transpose(out=trb[:, j * 128:(j + 1) * 128],
                            in_=vsb0[:, j * 128:(j + 1) * 128],
                            identity=ident_bf)
    trv = trb.rearrange("p (j k) -> p j k", k=128)
    nc.vector.tensor_copy(out=vt_view[:, 0:8, 64:128], in_=trv[:, 0:8, 0:64])
    nc.vector.tensor_copy(out=vt_view[:, 0:8, 192:256], in_=trv[:, 0:8, 64:128])
    nc.tensor.transpose(out=trb[:, 1024:1152], in_=aug_src, identity=ident_bf)
    aug_stage = php.tile([128, 128], BF16, name="aug_stage")
    nc.vector.tensor_copy(out=aug_stage, in_=trb[:, 1024:1152])
    aug_rows = ((ka[0], 64), (ka[0], 65), (qa[0], 64), (qa[0], 65),
                (ka[1], 64), (ka[1], 65), (qa[1], 64), (qa[1], 65))
    for v, (tgt, row) in enumerate(aug_rows):
        q = nc.sync if v % 2 == 0 else nc.gpsimd
        q.dma_start(
            out=tgt[row:row + 1, :].rearrange("a (b c) -> a b c", b=LT),
            in_=aug_stage[v * 16:(v + 1) * 16, :])

    # ================= attention =================
    # ot-tag rotation: q0ps(1) pT0ps(2) oT00(1) vT1ps(2) oT01(1) q1ps(2)
    #                  oT10(1) opc0(2) oT11(1) opc1(2)
    vT1ps = None
    vsb1 = None
    q1ps = None
    opc0 = None
    trb2 = None

    def outproj_tile(c, lt, opps, tail):
        col = (c * 8 + lt) * 128
        _mmg(nc, oT_sb[:, col:col + 128],
             [(opps[:, n * 512:(n + 1) * 512],
               wo_sb[:, n * 512:(n + 1) * 512]) for n in range(2)],
             start=True, stop=True)
        ob = osb.tile([128, D], BF16, tag="osb", bufs=4, name=f"ob{c}{lt}")
        if tail:
            # post-attention: ACT is free — split the evac across engines
            nc.scalar.activation(out=ob[:, 0:512], in_=opps[:, 0:512],
                                 func=AF.Copy)
            nc.vector.tensor_copy(out=ob[:, 512:1024], in_=opps[:, 512:1024])
        else:
            nc.vector.tensor_copy(out=ob, in_=opps)
        nc.sync.dma_start(out=out_d[col:col + 128, :], in_=ob)

    for c, h in ((0, 0), (0, 1), (1, 0), (1, 1)):
        oTps = ps.tile([128, 1024], FP32, tag="ot", bufs=2, name=f"oT{c}{h}")
        lo = 128 * h   # head block [ones | v_h] within the 256-wide lt group
        deferred_av = []
        for lk in range(LT):
            stps = ps.tile([128, 1024], FP32, tag="st", bufs=2,
                           name=f"st{c}{h}{lk}")
            _mmg(nc, ka[h][0:66, lk * 128:(lk + 1) * 128],
                 [(stps[:, n * 512:(n + 1) * 512],
                   qa[h][0:66, c * 1024 + n * 512: c * 1024 + (n + 1) * 512])
                  for n in range(2)], start=True, stop=True)
            ex = expp.tile([128, 1024], BF16, tag="exp", bufs=4,
                           name=f"ex{c}{h}{lk}")
            nc.scalar.activation(out=ex, in_=stps, func=AF.Exp)

            def emit_av(lk, ex):
                _mmg(nc, vt[:, lk * 256 + lo: lk * 256 + lo + 128],
                     [(oTps[:, n * 512:(n + 1) * 512],
                       ex[:, n * 512:(n + 1) * 512]) for n in range(2)],
                     start=(lk == 0), stop=(lk == LT - 1))

            if lk < (3 if (c, h) == (0, 0) else 2):
                # at (0,0) the v tiles arrive slightly after the aug rows;
                # at later segments the first A@V waits the previous oT
                # drain — let scores/exps run ahead, flush the A@Vs after
                deferred_av.append((lk, ex))
            else:
                for a_lk, a_ex in deferred_av:
                    emit_av(a_lk, a_ex)
                deferred_av = []
                emit_av(lk, ex)

            # -------- injected work --------
            if (c, h) == (0, 0):
                if lk < 4:   # vT half 1: two d-chunks per lk
                    if lk == 0:
                        vT1ps = ps.tile([128, 1024], FP32, tag="ot", bufs=2,
                                        name="vT1ps")
                    for dc in (2 * lk, 2 * lk + 1):
                        _mmg(nc, wv_sb[:, dc * 128:(dc + 1) * 128],
                             [(vT1ps[:, n * 512:(n + 1) * 512],
                               xT[dc][:, 1024 + n * 512:1024 + (n + 1) * 512])
                              for n in range(2)],
                             start=(dc == 0), stop=(dc == DT - 1))
                elif lk == 4:
                    vsb1 = vtp.tile([128, 1024], BF16, tag="vsb", bufs=2,
                                    name="vsb1")
                    nc.vector.tensor_scalar(out=vsb1, in0=vT1ps,
                                            scalar1=b4[:, 2:3], scalar2=None,
                                            op0=ALU.add)
                elif lk in (5, 6):
                    if lk == 5:
                        trb2 = ps.tile([128, 2048], BF16, tag="st", bufs=2,
                                       name="trb2")
                    js = range(0, 4) if lk == 5 else range(4, 8)
                    for j in js:
                        nc.tensor.transpose(
                            out=trb2[:, j * 128:(j + 1) * 128],
                            in_=vsb1[:, j * 128:(j + 1) * 128],
                            identity=ident_bf)
                    if lk == 6:
                        trv2 = trb2.rearrange("p (j k) -> p j k", k=128)
                        nc.vector.tensor_copy(out=vt_view[:, 8:16, 64:128],
                                              in_=trv2[:, 0:8, 0:64])
                        nc.vector.tensor_copy(out=vt_view[:, 8:16, 192:256],
                                              in_=trv2[:, 0:8, 64:128])
            elif (c, h) == (0, 1):
                if lk < 8:   # q half 1: one d-chunk per lk
                    if lk == 0:
                        q1ps = ps.tile([128, 1024], FP32, tag="ot", bufs=2,
                                       name="q1ps")
                    _mmg(nc, wq_sb[:, lk * 128:(lk + 1) * 128],
                         [(q1ps[:, n * 512:(n + 1) * 512],
                           xT[lk][:, 1024 + n * 512:1024 + (n + 1) * 512])
                          for n in range(2)],
                         start=(lk == 0), stop=(lk == DT - 1))
                elif lk in (8, 9):
                    hh = lk - 8
                    nc.vector.tensor_scalar(
                        out=qa[hh][0:HD, 1024:2048],
                        in0=q1ps[hh * HD:(hh + 1) * HD, :],
                        scalar1=b4[hh * HD:(hh + 1) * HD, 0:1], scalar2=None,
                        op0=ALU.add)
            elif (c, h) == (1, 0):
                if lk % 2 == 1:   # out-proj of half 0, one l-tile per 2 lk
                    if lk == 1:
                        opc0 = ps.tile([128, 1024], FP32, tag="ot", bufs=2,
                                       name="opc0")
                    outproj_tile(0, (lk - 1) // 2, opc0, tail=False)

        # -------- drain: softmax normalize into oT_sb (bf16) --------
        # reciprocal_approx_fast is a custom DVE op and silently misreads
        # partition-shifted operands: keep it at partition base 0 (the vt
        # layout puts denominator rows at psum base 0 for both heads).
        rv = otp.tile([64, 1024], FP32, tag="rv", bufs=2, name=f"rv{c}{h}")
        if (c, h) == (1, 1):
            # split the last drain so the tail out-proj starts sooner
            for nn in range(2):
                s = slice(nn * 512, (nn + 1) * 512)
                nc.vector.reciprocal_approx_fast(out=rv[:, s],
                                                 in_=oTps[0:64, s])
                nc.vector.tensor_tensor(
                    out=oT_sb[h * 64:(h + 1) * 64,
                              c * 1024 + nn * 512:c * 1024 + (nn + 1) * 512],
                    in0=oTps[64:128, s], in1=rv[:, s], op=ALU.mult)
        else:
            nc.vector.reciprocal_approx_fast(out=rv, in_=oTps[0:64, :])
            nc.vector.tensor_tensor(
                out=oT_sb[h * 64:(h + 1) * 64, c * 1024:(c + 1) * 1024],
                in0=oTps[64:128, :], in1=rv, op=ALU.mult)

    # ---- tail: out-proj of half 1 (two tiles, pipelined) ----
    opc1a = ps.tile([128, 1024], FP32, tag="ot", bufs=2, name="opc1a")
    opc1b = ps.tile([128, 1024], FP32, tag="ot", bufs=2, name="opc1b")
    for lt in range(8):
        outproj_tile(1, lt, opc1a if lt % 2 == 0 else opc1b, tail=True)

    if dbg:
        nc.sync.dma_start(out=dbg["d_ka0"], in_=ka[0])
        nc.sync.dma_start(out=dbg["d_ka1"], in_=ka[1])
        nc.sync.dma_start(out=dbg["d_qa0"], in_=qa[0])
        nc.sync.dma_start(out=dbg["d_qa1"], in_=qa[1])
        nc.sync.dma_start(out=dbg["d_vt"], in_=vt)
        nc.sync.dma_start(out=dbg["d_oT"], in_=oT_sb)
        nc.sync.dma_start(out=dbg["d_pT"], in_=pT_sb)
        nc.sync.dma_start(out=dbg["d_asrc"], in_=aug_src)
        nc.sync.dma_start(out=dbg["d_astg"], in_=aug_stage)
    ctx.close()


def _get_nc():
    global _NC
    if _NC is None:
        _NC = _build()
    return _NC


def kernel(x, Wq, bq, Wk, bk, Wv, bv, Wo, bo, Wp, bp, gamma):
    global LAST_EXEC_NS, LAST_RESULTS
    import ml_dtypes
    BF = ml_dtypes.bfloat16
    nc = _get_nc()

    x2 = np.asarray(x, np.float32).reshape(L, D)
    xt = np.ascontiguousarray(x2.T).astype(BF)
    Wq = np.asarray(Wq, np.float32)
    Wk = np.asarray(Wk, np.float32)
    Wv = np.asarray(Wv, np.float32)
    Wo = np.asarray(Wo, np.float32)
    Wp = np.asarray(Wp, np.float32)
    bq_ = np.asarray(bq, np.float32)
    bk_ = np.asarray(bk, np.float32)
    bv_ = np.asarray(bv, np.float32)
    bp_ = np.asarray(bp, np.float32)
    g_all = (1.0 / (1.0 + np.exp(-np.asarray(gamma, np.float32)))) * BETA
    sc = 1.0 / np.sqrt(HD)

    in_maps = []
    for c in range(N_CORES):
        hs = slice(c * HW, (c + 1) * HW)
        p4 = slice(c * 2 * NH, (c + 1) * 2 * NH)
        bias4 = np.zeros((128, 4), np.float32)
        bias4[:, 0] = bq_[hs] * sc
        bias4[:, 1] = bk_[hs]
        bias4[:, 2] = bv_[hs]
        bias4[0:2 * NH, 3] = bp_[p4]
        garr = np.repeat(g_all[c * NH:(c + 1) * NH][None, :], 128, axis=0)

        def chunk(wT):   # [D, E] -> [128, DT*E]: chunk dc at cols dc*E..
            e = wT.shape[1]
            return np.ascontiguousarray(
                wT.reshape(DT, 128, e).transpose(1, 0, 2).reshape(128, DT * e)
            ).astype(BF)

        in_maps.append({
            "xt": xt,
            "wq": chunk((Wq[hs] * sc).T),
            "wk": chunk(Wk[hs].T),
            "wv": chunk(Wv[hs].T),
            "wp": chunk(Wp[p4].T),
            "wo": np.ascontiguousarray(Wo[:, hs].T).astype(BF),
            "bias4": bias4,
            "garr": np.ascontiguousarray(garr),
        })
    res = run_bass_kernel_spmd(nc, in_maps, list(range(N_CORES)), trace=TRACE)
    LAST_EXEC_NS = res.exec_time_ns
    LAST_RESULTS = res
    acc = np.zeros((L, D), np.float32)
    for c in range(N_CORES):
        acc += np.asarray(res.results[c]["partial"], dtype=np.float32)
    acc += np.asarray(bo, np.float32)[None, :]
    return acc.reshape(B, L, D)
